# revision 26
# baseline (speedup 1.0000x reference)
"""Multi-head attention + output projection, sharded over 8 TRN2 NeuronCores.

Problem: Q,K,V [4,1024,1024] f32; 16 heads x 64 dim; softmax(QK^T/sqrt(1024))V,
concat heads, out @ W_H.T + b_H.

Sharding: 8 cores = 4 batch x 2 query-halves. Each core computes full attention
(all 16 heads, all 1024 keys) for its 512 queries plus the output projection for
those rows. Output rows are disjoint -> no collectives.

v2 kernel (bf16 matmul datapath):
  - All matmul operands bf16 (fp32 streams at half rate on the PE; bf16 also
    enables fast weight load). PSUM accumulation stays fp32.
  - Heads processed in pairs j=(2j,2j+1). QK^T for the two heads uses PE row
    groups 0-63 / 64-127 (K=64 row tiling) so the two matmuls run concurrently.
  - exp on ACT reads both heads' scores for one key-chunk from PSUM in a single
    [128,1024] ACTIVATE, writing bf16.
  - attn*V accumulates [65,512] per head (V augmented with a ones column ->
    row 64 = softmax denominator).
  - normalize: accumulators are evacuated to SBUF in fp32; 1/denom via the
    single-op custom-DVE reciprocal_approx_fast (fp32, ~51 ULP), keeping the
    reciprocal off the busy ACT engine. The reciprocal row is broadcast
    across 64 partitions with a K=1 outer-product matmul, then one DVE
    multiply per head produces the bf16 normalized outT (DVE tensor ops
    handle the odd head's partition-shifted output directly).
  - projection: 64 bf16 matmuls contracting the 1024 concat dims in 8 chunks;
    the cc=0..3 half runs interleaved with head pairs 4..7 (PE slack), with
    bias folded into its PSUM->SBUF evacuation; the cc=4..7 half plus a
    combine add runs after the last pair.
"""
import sys
import os

sys.path.insert(0, "/opt/trn_rl_repo")

import numpy as np

B, L, D, H, HD = 4, 1024, 1024, 16, 64
NCORES = 8
QBLK = L // 2  # 512 queries per core
NPAIR = H // 2
SCALE = 1.0 / np.sqrt(np.float32(D))

_STATE = {}


def _patch_act_table_loads(nc):
    """Rewrite the activation-table loads to use the combined
    natural_log_exp_and_others set (covers both Exp and Ln) and drop the
    redundant re-loads the first-match inserter generated."""
    from concourse.hw_specs import get_activation_tables

    tabs = get_activation_tables(nc.m.arch)
    names = list(tabs.keys())
    combined_id = names.index("natural_log_exp_and_others")
    for f in nc.m.functions:
        for blk in f.blocks:
            insts = list(blk.instructions)
            keep, seen, dropped = [], False, 0
            for inst in insts:
                if type(inst).__name__ == "InstLoadActFuncSet":
                    assert not (inst.has_wait() or inst.has_update())
                    if not seen:
                        inst.act_func_set_id = combined_id
                        seen = True
                        keep.append(inst)
                    else:
                        dropped += 1
                else:
                    keep.append(inst)
            if dropped:
                blk.instructions = keep


def _build_nc():
    import concourse.bass as bass
    import concourse.tile as tile
    from concourse import bacc, mybir
    from contextlib import ExitStack

    F32 = mybir.dt.float32
    BF16 = mybir.dt.bfloat16
    Exp = mybir.ActivationFunctionType.Exp

    F32 = mybir.dt.float32
    Identity = mybir.ActivationFunctionType.Identity

    nc = bacc.Bacc("TRN2", target_bir_lowering=False, debug=False, use_seq_codegen=True)
    qt = nc.dram_tensor("qt", [128, NPAIR, QBLK], BF16, kind="ExternalInput")
    kt = nc.dram_tensor("kt", [128, NPAIR, L], BF16, kind="ExternalInput")
    vv = nc.dram_tensor("vv", [128, H, 8, HD + 1], BF16, kind="ExternalInput")
    wht = nc.dram_tensor("wht", [128, 8, D], BF16, kind="ExternalInput")
    bias = nc.dram_tensor("bias", [1, D], BF16, kind="ExternalInput")
    out = nc.dram_tensor("out", [QBLK, D], F32, kind="ExternalOutput")

    with tile.TileContext(nc) as tc, ExitStack() as ctx:
        singles = ctx.enter_context(tc.tile_pool(name="singles", bufs=1))
        v_pool = ctx.enter_context(tc.tile_pool(name="vp", bufs=4))
        exp_pool = ctx.enter_context(tc.tile_pool(name="ep", bufs=2))
        ovs_pool = ctx.enter_context(tc.tile_pool(name="ovs", bufs=2))
        rc_pool = ctx.enter_context(tc.tile_pool(name="rc", bufs=2))
        fo_pool = ctx.enter_context(tc.tile_pool(name="fo", bufs=2))
        scps = ctx.enter_context(tc.tile_pool(name="scps", bufs=2, space="PSUM"))
        ovps = ctx.enter_context(tc.tile_pool(name="ovps", bufs=2, space="PSUM"))

        # warm the exp/ln table while DMAs run
        warm_in = singles.tile([1, 8], F32, tag="warm_in")
        warm_out = singles.tile([1, 8], F32, tag="warm_out")
        nc.vector.memset(warm_in, 0.5)
        nc.scalar.activation(out=warm_out, in_=warm_in, func=Exp)

        ones_row = singles.tile([1, 128], BF16, tag="ones_row")
        nc.vector.memset(ones_row, 1.0)

        qt_sb = singles.tile([128, NPAIR, QBLK], BF16, tag="qt")
        kt_sb = singles.tile([128, NPAIR, L], BF16, tag="kt")
        outT = singles.tile([128, NPAIR, QBLK], BF16, tag="outT")
        sb_bias = singles.tile([1, D], BF16, tag="bias")
        sb_wht = singles.tile([128, 8, D], BF16, tag="wht")

        v_tiles = []
        for j in range(NPAIR):
            if j == 0:
                # land the first score matmul's kt chunk ASAP
                nc.sync.dma_start(kt_sb[:, 0, 0:128], kt.ap()[:, 0, 0:128])
                nc.sync.dma_start(qt_sb[:, 0, :], qt.ap()[:, 0])
                nc.sync.dma_start(kt_sb[:, 0, 128:L], kt.ap()[:, 0, 128:L])
            else:
                nc.sync.dma_start(qt_sb[:, j, :], qt.ap()[:, j])
                nc.sync.dma_start(kt_sb[:, j, :], kt.ap()[:, j])
            ve = v_pool.tile([128, 8, HD + 1], BF16, tag="ve")
            nc.sync.dma_start(ve, vv.ap()[:, 2 * j])
            vo = v_pool.tile([128, 8, HD + 1], BF16, tag="vo")
            nc.sync.dma_start(vo, vv.ap()[:, 2 * j + 1])
            v_tiles.append((ve, vo))
        nc.sync.dma_start(sb_bias, bias.ap())
        for cc in range(8):
            nc.sync.dma_start(sb_wht[:, cc], wht.ap()[:, cc])

        proj_tiles = [(m, jn) for m in range(QBLK // 128) for jn in range(D // 512)]
        norm_state = {}  # j -> (ovs, recip)

        def emit_normalize(jj):
            """bc broadcast + normalize TTs for pair jj (emitted deferred, so
            the PE queue never blocks on the DVE recip chain); the
            broadcast runs on the idle GpSimd engine."""
            ovs, recip = norm_state.pop(jj)
            bc = rc_pool.tile([HD, 2, QBLK], F32, tag="bcs", name=f"bc{jj}")
            nc.gpsimd.partition_broadcast(bc, recip)
            nc.vector.tensor_mul(
                out=outT[0:HD, jj, :], in0=ovs[0:HD, 0, :], in1=bc[:, 0, :])
            nc.vector.tensor_mul(
                out=outT[HD:128, jj, :], in0=ovs[0:HD, 1, :], in1=bc[:, 1, :])

        for j in range(NPAIR):
            ve, vo = v_tiles[j]
            # epair[p, c, eo, q]
            epair = exp_pool.tile([128, 8, 2, QBLK], BF16, tag="epair")
            ov = ovps.tile([HD + 1, 2, QBLK], F32, tag="ov")

            def attnv(c):
                nc.tensor.matmul(
                    ov[:, 0, :], lhsT=ve[:, c, :], rhs=epair[:, c, 0, :],
                    start=(c == 0), stop=(c == 7))
                nc.tensor.matmul(
                    ov[:, 1, :], lhsT=vo[:, c, :], rhs=epair[:, c, 1, :],
                    start=(c == 0), stop=(c == 7))

            for c in range(8):
                S = scps.tile([128, 2, QBLK], F32, tag="S")
                # two heads' QK^T on complementary PE row groups (K=64)
                nc.tensor.matmul(
                    S[:, 0, :],
                    lhsT=kt_sb[0:HD, j, c * 128:(c + 1) * 128],
                    rhs=qt_sb[0:HD, j, :],
                    start=True, stop=True)
                nc.tensor.matmul(
                    S[:, 1, :],
                    lhsT=kt_sb[HD:128, j, c * 128:(c + 1) * 128],
                    rhs=qt_sb[HD:128, j, :],
                    start=True, stop=True)
                nc.scalar.activation(out=epair[:, c, :, :], in_=S, func=Exp)
                if c >= 1:
                    attnv(c - 1)
                if c == 3 and j >= 1:
                    emit_normalize(j - 1)
            attnv(7)

            # evacuate accumulators (fp32); row 64 of each = softmax denominator
            ovs = ovs_pool.tile([HD + 1, 2, QBLK], F32, tag="ovs")
            nc.vector.tensor_copy(out=ovs, in_=ov)
            # 1/denom on DVE (single custom op, fp32, ~51 ULP). The custom op
            # mishandles nonzero base partitions, so stage the denominator row
            # at partition 0 first.
            den = rc_pool.tile([1, 2, QBLK], F32, tag="den")
            nc.vector.tensor_copy(out=den, in_=ovs[HD:HD + 1, :, :])
            recip = rc_pool.tile([1, 2, QBLK], F32, tag="recip")
            nc.vector.reciprocal_approx_fast(out=recip, in_=den)
            norm_state[j] = (ovs, recip)

        # keep the PE busy across the last pair's reciprocal window so HAM
        # does not re-throttle the projection to 1.2 GHz
        junk = scps.tile([128, 2, QBLK], F32, tag="S", name="junk")
        for u in range(12):
            nc.tensor.matmul(
                junk[:, u % 2, :], lhsT=sb_wht[:, 0, 0:128],
                rhs=sb_wht[:, 1, 0:512], start=True, stop=True)

        emit_normalize(NPAIR - 1)

        # tail: full projection, 3 "S" PSUM slots, bias folded in as a K=1
        # matmul, cc=7 (the last-normalized pair) last in each chain;
        # evacuation alternates between the idle ACT and DVE
        for idx, (m, jn) in enumerate(proj_tiles):
            pool, tag = (scps, "S") if idx % 2 == 0 else (ovps, "ov")
            P = pool.tile([128, 512], F32, tag=tag, name=f"p{idx}")
            nc.tensor.matmul(
                P, lhsT=ones_row, rhs=sb_bias[:, jn * 512:(jn + 1) * 512],
                start=True, stop=False)
            for cc in range(8):
                nc.tensor.matmul(
                    P,
                    lhsT=outT[:, cc, m * 128:(m + 1) * 128],
                    rhs=sb_wht[:, cc, jn * 512:(jn + 1) * 512],
                    start=False, stop=(cc == 7))
            Fo = fo_pool.tile([128, 512], F32, tag="F")
            if idx % 2 == 0:
                nc.scalar.activation(out=Fo, in_=P, func=Identity)
            else:
                nc.vector.tensor_copy(out=Fo, in_=P)
            nc.gpsimd.dma_start(
                out.ap()[m * 128:(m + 1) * 128, jn * 512:(jn + 1) * 512],
                Fo)

    nc.compile()
    _patch_act_table_loads(nc)
    return nc


def _host_shard(Q, K, V, W_H, b_H):
    """Build the 8 per-core input dicts (all host-side numpy)."""
    import ml_dtypes
    bf16 = ml_dtypes.bfloat16

    Qs = (np.asarray(Q, np.float32) * SCALE).astype(bf16)
    K = np.asarray(K, np.float32).astype(bf16)
    V = np.asarray(V, np.float32).astype(bf16)
    W_H = np.asarray(W_H, np.float32).astype(bf16)
    b_H = np.asarray(b_H, np.float32)

    # [hd, n] chunked: [128, 8, D]
    wht = np.ascontiguousarray(W_H.T.reshape(8, 128, D).transpose(1, 0, 2))
    bias = np.asarray(b_H, np.float32).reshape(1, D).astype(bf16)

    in_maps = []
    for c in range(NCORES):
        b, half = divmod(c, 2)
        qlo = half * QBLK
        # [q, j, par, d] -> [par, d, j, q] -> [128, NPAIR, QBLK]
        qtc = np.ascontiguousarray(
            Qs[b, qlo:qlo + QBLK].reshape(QBLK, NPAIR, 2, HD).transpose(2, 3, 1, 0)
        ).reshape(128, NPAIR, QBLK)
        ktc = np.ascontiguousarray(
            K[b].reshape(L, NPAIR, 2, HD).transpose(2, 3, 1, 0)
        ).reshape(128, NPAIR, L)
        # V_aug [k, h, 65] -> [c, p, h, e] -> [p, h, c, e]
        va = np.concatenate(
            [V[b].reshape(L, H, HD), np.ones((L, H, 1), bf16)], axis=2)
        vvc = np.ascontiguousarray(
            va.reshape(8, 128, H, HD + 1).transpose(1, 2, 0, 3))
        in_maps.append({"qt": qtc, "kt": ktc, "vv": vvc, "wht": wht,
                        "bias": bias})
    return in_maps


def _get_runner():
    """Build (once) and cache a jitted 8-core runner for the kernel."""
    if "runner" in _STATE:
        return _STATE["runner"]

    import jax
    from jax.sharding import Mesh, PartitionSpec, NamedSharding
    from jax.experimental.shard_map import shard_map
    from concourse import bass2jax, mybir

    nc = _build_nc()
    bass2jax.install_neuronx_cc_hook()

    partition_name = (
        nc.partition_id_tensor.name if nc.partition_id_tensor else None)
    in_names, out_names, out_avals, zero_shapes = [], [], [], []
    for alloc in nc.m.functions[0].allocations:
        if not isinstance(alloc, mybir.MemoryLocationSet):
            continue
        name = alloc.memorylocations[0].name
        if alloc.kind == "ExternalInput":
            if name != partition_name:
                in_names.append(name)
        elif alloc.kind == "ExternalOutput":
            out_names.append(name)
            shape = tuple(alloc.tensor_shape)
            dtype = mybir.dt.np(alloc.dtype)
            out_avals.append(jax.core.ShapedArray(shape, dtype))
            zero_shapes.append((shape, dtype))
    n_params = len(in_names)
    n_outs = len(out_avals)
    all_names = list(in_names) + list(out_names)
    if partition_name is not None:
        all_names.append(partition_name)
    donate = tuple(range(n_params, n_params + n_outs))

    def _body(*args):
        operands = list(args)
        if partition_name is not None:
            operands.append(bass2jax.partition_id_tensor())
        outs = bass2jax._bass_exec_p.bind(
            *operands,
            out_avals=tuple(out_avals),
            in_names=tuple(all_names),
            out_names=tuple(out_names),
            lowering_input_output_aliases=(),
            sim_require_finite=True,
            sim_require_nnan=True,
            nc=nc,
        )
        return tuple(outs)

    devices = jax.devices()[:NCORES]
    mesh = Mesh(np.asarray(devices), ("core",))
    in_specs = (PartitionSpec("core"),) * (n_params + n_outs)
    out_specs = (PartitionSpec("core"),) * n_outs
    sharded = jax.jit(
        shard_map(_body, mesh=mesh, in_specs=in_specs, out_specs=out_specs,
                  check_rep=False),
        donate_argnums=donate,
        keep_unused=True,
    )
    sharding = NamedSharding(mesh, PartitionSpec("core"))

    def put_inputs(in_maps):
        return [
            jax.device_put(
                np.concatenate(
                    [np.asarray(in_maps[c][nm]) for c in range(NCORES)], axis=0),
                sharding)
            for nm in in_names
        ]

    def run(in_maps, device_inputs=None):
        if device_inputs is None:
            device_inputs = put_inputs(in_maps)
        zeros = [
            jax.device_put(np.zeros((NCORES * s[0], *s[1:]), d), sharding)
            for s, d in zero_shapes
        ]
        out_arrs = sharded(*device_inputs, *zeros)
        results = []
        for c in range(NCORES):
            results.append({
                name: np.asarray(out_arrs[i]).reshape(
                    NCORES, *out_avals[i].shape)[c]
                for i, name in enumerate(out_names)
            })
        return results

    runner = {"run": run, "put_inputs": put_inputs, "sharded": sharded,
              "in_names": in_names, "out_names": out_names,
              "zero_shapes": zero_shapes, "nc": nc}
    _STATE["runner"] = runner
    return runner


def kernel(Q=None, K=None, V=None, W_H=None, b_H=None, mask=None, **kw):
    in_maps = _host_shard(Q, K, V, W_H, b_H)
    runner = _get_runner()
    results = runner["run"](in_maps)
    out = np.empty((B, L, D), np.float32)
    for c in range(NCORES):
        b, half = divmod(c, 2)
        out[b, half * QBLK:(half + 1) * QBLK, :] = results[c]["out"]
    return out


# revision 27
# speedup vs baseline: 1.0521x; 1.0521x over previous
"""Multi-head attention + output projection, sharded over 8 TRN2 NeuronCores.

Problem: Q,K,V [4,1024,1024] f32; 16 heads x 64 dim; softmax(QK^T/sqrt(1024))V,
concat heads, out @ W_H.T + b_H.

Sharding: 8 cores = 4 batch x 2 query-halves. Each core computes full attention
(all 16 heads, all 1024 keys) for its 512 queries plus the output projection for
those rows. Output rows are disjoint -> no collectives.

v2 kernel (bf16 matmul datapath):
  - All matmul operands bf16 (fp32 streams at half rate on the PE; bf16 also
    enables fast weight load). PSUM accumulation stays fp32.
  - Heads processed in pairs j=(2j,2j+1). QK^T for the two heads uses PE row
    groups 0-63 / 64-127 (K=64 row tiling) so the two matmuls run concurrently.
  - exp on ACT reads both heads' scores for one key-chunk from PSUM in a single
    [128,1024] ACTIVATE, writing bf16.
  - attn*V accumulates [65,512] per head (V augmented with a ones column ->
    row 64 = softmax denominator).
  - normalize: accumulators are evacuated to SBUF in fp32; 1/denom via the
    single-op custom-DVE reciprocal_approx_fast (fp32, ~51 ULP), keeping the
    reciprocal off the busy ACT engine. The reciprocal row is broadcast
    across 64 partitions with a K=1 outer-product matmul, then one DVE
    multiply per head produces the bf16 normalized outT (DVE tensor ops
    handle the odd head's partition-shifted output directly).
  - projection: 64 bf16 matmuls contracting the 1024 concat dims in 8 chunks;
    the cc=0..3 half runs interleaved with head pairs 4..7 (PE slack), with
    bias folded into its PSUM->SBUF evacuation; the cc=4..7 half plus a
    combine add runs after the last pair.
"""
import sys
import os

sys.path.insert(0, "/opt/trn_rl_repo")

import numpy as np

B, L, D, H, HD = 4, 1024, 1024, 16, 64
NCORES = 8
QBLK = L // 2  # 512 queries per core
NPAIR = H // 2
SCALE = 1.0 / np.sqrt(np.float32(D))

_STATE = {}


def _patch_act_table_loads(nc):
    """Rewrite the activation-table loads to use the combined
    natural_log_exp_and_others set (covers both Exp and Ln) and drop the
    redundant re-loads the first-match inserter generated."""
    from concourse.hw_specs import get_activation_tables

    tabs = get_activation_tables(nc.m.arch)
    names = list(tabs.keys())
    combined_id = names.index("natural_log_exp_and_others")
    for f in nc.m.functions:
        for blk in f.blocks:
            insts = list(blk.instructions)
            keep, seen, dropped = [], False, 0
            for inst in insts:
                if type(inst).__name__ == "InstLoadActFuncSet":
                    assert not (inst.has_wait() or inst.has_update())
                    if not seen:
                        inst.act_func_set_id = combined_id
                        seen = True
                        keep.append(inst)
                    else:
                        dropped += 1
                else:
                    keep.append(inst)
            if dropped:
                blk.instructions = keep


def _build_nc():
    import concourse.bass as bass
    import concourse.tile as tile
    from concourse import bacc, mybir
    from contextlib import ExitStack

    F32 = mybir.dt.float32
    BF16 = mybir.dt.bfloat16
    Exp = mybir.ActivationFunctionType.Exp

    F32 = mybir.dt.float32
    Identity = mybir.ActivationFunctionType.Identity

    nc = bacc.Bacc("TRN2", target_bir_lowering=False, debug=False, use_seq_codegen=True)
    qt = nc.dram_tensor("qt", [128, NPAIR, QBLK], BF16, kind="ExternalInput")
    kt = nc.dram_tensor("kt", [128, NPAIR, L], BF16, kind="ExternalInput")
    vv = nc.dram_tensor("vv", [128, H, 8, HD + 1], BF16, kind="ExternalInput")
    wht = nc.dram_tensor("wht", [128, 8, D], BF16, kind="ExternalInput")
    bias = nc.dram_tensor("bias", [1, D], BF16, kind="ExternalInput")
    out = nc.dram_tensor("out", [QBLK, D], F32, kind="ExternalOutput")

    with tile.TileContext(nc) as tc, ExitStack() as ctx:
        singles = ctx.enter_context(tc.tile_pool(name="singles", bufs=1))
        v_pool = ctx.enter_context(tc.tile_pool(name="vp", bufs=4))
        exp_pool = ctx.enter_context(tc.tile_pool(name="ep", bufs=2))
        ovs_pool = ctx.enter_context(tc.tile_pool(name="ovs", bufs=2))
        rc_pool = ctx.enter_context(tc.tile_pool(name="rc", bufs=2))
        fo_pool = ctx.enter_context(tc.tile_pool(name="fo", bufs=2))
        scps = ctx.enter_context(tc.tile_pool(name="scps", bufs=3, space="PSUM"))
        ovps = ctx.enter_context(tc.tile_pool(name="ovps", bufs=1, space="PSUM"))

        # warm the exp/ln table while DMAs run
        warm_in = singles.tile([1, 8], F32, tag="warm_in")
        warm_out = singles.tile([1, 8], F32, tag="warm_out")
        nc.vector.memset(warm_in, 0.5)
        nc.scalar.activation(out=warm_out, in_=warm_in, func=Exp)

        ones_row = singles.tile([1, 128], BF16, tag="ones_row")
        nc.vector.memset(ones_row, 1.0)

        qt_sb = singles.tile([128, NPAIR, QBLK], BF16, tag="qt")
        kt_sb = singles.tile([128, NPAIR, L], BF16, tag="kt")
        outT = singles.tile([128, NPAIR, QBLK], BF16, tag="outT")
        sb_bias = singles.tile([1, D], BF16, tag="bias")
        sb_wht = singles.tile([128, 8, D], BF16, tag="wht")

        v_tiles = []
        for j in range(NPAIR):
            if j == 0:
                # land the first score matmul's kt chunk ASAP
                nc.sync.dma_start(kt_sb[:, 0, 0:128], kt.ap()[:, 0, 0:128])
                nc.sync.dma_start(qt_sb[:, 0, :], qt.ap()[:, 0])
                nc.sync.dma_start(kt_sb[:, 0, 128:L], kt.ap()[:, 0, 128:L])
            else:
                nc.sync.dma_start(qt_sb[:, j, :], qt.ap()[:, j])
                nc.sync.dma_start(kt_sb[:, j, :], kt.ap()[:, j])
            vt = v_pool.tile([128, 2, 8, HD + 1], BF16, tag="vt")
            nc.sync.dma_start(vt, vv.ap()[:, 2 * j:2 * j + 2])
            v_tiles.append(vt)
        nc.sync.dma_start(sb_bias, bias.ap())
        for cc in range(8):
            nc.sync.dma_start(sb_wht[:, cc], wht.ap()[:, cc])

        # warm HAM during the input-DMA ramp: matmuls on constant tiles
        wj_w = singles.tile([128, 128], BF16, tag="wj_w")
        nc.vector.memset(wj_w, 0.25)
        wj_r = singles.tile([128, 512], BF16, tag="wj_r")
        nc.vector.memset(wj_r, 0.25)
        sjunk = scps.tile([128, 2, QBLK], F32, tag="S", name="sjunk")
        for u in range(12):
            nc.tensor.matmul(sjunk[:, u % 2, :], lhsT=wj_w, rhs=wj_r,
                             start=True, stop=True)

        proj_tiles = [(m, jn) for m in range(QBLK // 128) for jn in range(D // 512)]
        norm_state = {}  # j -> (ovs, recip)

        def emit_normalize(jj):
            """bc broadcast + normalize TTs for pair jj (emitted deferred, so
            the PE queue never blocks on the DVE recip chain); the
            broadcast runs on the idle GpSimd engine."""
            ovs, recip = norm_state.pop(jj)
            bc = rc_pool.tile([HD, 2, QBLK], F32, tag="bcs", name=f"bc{jj}")
            nc.gpsimd.partition_broadcast(bc, recip)
            nc.vector.tensor_mul(
                out=outT[0:HD, jj, :], in0=ovs[0:HD, 0, :], in1=bc[:, 0, :])
            nc.vector.tensor_mul(
                out=outT[HD:128, jj, :], in0=ovs[0:HD, 1, :], in1=bc[:, 1, :])

        for j in range(NPAIR):
            vt = v_tiles[j]
            # epair[p, c, eo, q]
            epair = exp_pool.tile([128, 8, 2, QBLK], BF16, tag="epair")
            ov = ovps.tile([HD + 1, 2, QBLK], F32, tag="ov")

            def attnv(c):
                nc.tensor.matmul(
                    ov[:, 0, :], lhsT=vt[:, 0, c, :], rhs=epair[:, c, 0, :],
                    start=(c == 0), stop=(c == 7))
                nc.tensor.matmul(
                    ov[:, 1, :], lhsT=vt[:, 1, c, :], rhs=epair[:, c, 1, :],
                    start=(c == 0), stop=(c == 7))

            for c in range(8):
                S = scps.tile([128, 2, QBLK], F32, tag="S")
                # two heads' QK^T on complementary PE row groups (K=64)
                nc.tensor.matmul(
                    S[:, 0, :],
                    lhsT=kt_sb[0:HD, j, c * 128:(c + 1) * 128],
                    rhs=qt_sb[0:HD, j, :],
                    start=True, stop=True)
                nc.tensor.matmul(
                    S[:, 1, :],
                    lhsT=kt_sb[HD:128, j, c * 128:(c + 1) * 128],
                    rhs=qt_sb[HD:128, j, :],
                    start=True, stop=True)
                nc.scalar.activation(out=epair[:, c, :, :], in_=S, func=Exp)
                if c >= 1:
                    attnv(c - 1)
                if c == 3 and j >= 1:
                    emit_normalize(j - 1)
            attnv(7)

            # evacuate accumulators (fp32); row 64 of each = softmax denominator
            ovs = ovs_pool.tile([HD + 1, 2, QBLK], F32, tag="ovs")
            nc.vector.tensor_copy(out=ovs, in_=ov)
            # 1/denom on DVE (single custom op, fp32, ~51 ULP). The custom op
            # mishandles nonzero base partitions, so stage the denominator row
            # at partition 0 first.
            den = rc_pool.tile([1, 2, QBLK], F32, tag="den")
            nc.vector.tensor_copy(out=den, in_=ovs[HD:HD + 1, :, :])
            recip = rc_pool.tile([1, 2, QBLK], F32, tag="recip")
            nc.vector.reciprocal_approx_fast(out=recip, in_=den)
            norm_state[j] = (ovs, recip)

        # keep the PE busy across the last pair's reciprocal window so HAM
        # does not re-throttle the projection to 1.2 GHz
        junk = scps.tile([128, 2, QBLK], F32, tag="S", name="junk")
        for u in range(12):
            nc.tensor.matmul(
                junk[:, u % 2, :], lhsT=sb_wht[:, 0, 0:128],
                rhs=sb_wht[:, 1, 0:512], start=True, stop=True)

        emit_normalize(NPAIR - 1)

        # tail: full projection, 3 "S" PSUM slots, bias folded in as a K=1
        # matmul, cc=7 (the last-normalized pair) last in each chain;
        # evacuation alternates between the idle ACT and DVE
        for idx, (m, jn) in enumerate(proj_tiles):
            P = scps.tile([128, 512], F32, tag="S", name=f"p{idx}")
            nc.tensor.matmul(
                P, lhsT=ones_row, rhs=sb_bias[:, jn * 512:(jn + 1) * 512],
                start=True, stop=False)
            for cc in range(8):
                nc.tensor.matmul(
                    P,
                    lhsT=outT[:, cc, m * 128:(m + 1) * 128],
                    rhs=sb_wht[:, cc, jn * 512:(jn + 1) * 512],
                    start=False, stop=(cc == 7))
            Fo = fo_pool.tile([128, 512], F32, tag="F")
            if idx % 2 == 0:
                nc.scalar.activation(out=Fo, in_=P, func=Identity)
            else:
                nc.vector.tensor_copy(out=Fo, in_=P)
            nc.gpsimd.dma_start(
                out.ap()[m * 128:(m + 1) * 128, jn * 512:(jn + 1) * 512],
                Fo)

    nc.compile()
    _patch_act_table_loads(nc)
    return nc


def _host_shard(Q, K, V, W_H, b_H):
    """Build the 8 per-core input dicts (all host-side numpy)."""
    import ml_dtypes
    bf16 = ml_dtypes.bfloat16

    Qs = (np.asarray(Q, np.float32) * SCALE).astype(bf16)
    K = np.asarray(K, np.float32).astype(bf16)
    V = np.asarray(V, np.float32).astype(bf16)
    W_H = np.asarray(W_H, np.float32).astype(bf16)
    b_H = np.asarray(b_H, np.float32)

    # [hd, n] chunked: [128, 8, D]
    wht = np.ascontiguousarray(W_H.T.reshape(8, 128, D).transpose(1, 0, 2))
    bias = np.asarray(b_H, np.float32).reshape(1, D).astype(bf16)

    in_maps = []
    for c in range(NCORES):
        b, half = divmod(c, 2)
        qlo = half * QBLK
        # [q, j, par, d] -> [par, d, j, q] -> [128, NPAIR, QBLK]
        qtc = np.ascontiguousarray(
            Qs[b, qlo:qlo + QBLK].reshape(QBLK, NPAIR, 2, HD).transpose(2, 3, 1, 0)
        ).reshape(128, NPAIR, QBLK)
        ktc = np.ascontiguousarray(
            K[b].reshape(L, NPAIR, 2, HD).transpose(2, 3, 1, 0)
        ).reshape(128, NPAIR, L)
        # V_aug [k, h, 65] -> [c, p, h, e] -> [p, h, c, e]
        va = np.concatenate(
            [V[b].reshape(L, H, HD), np.ones((L, H, 1), bf16)], axis=2)
        vvc = np.ascontiguousarray(
            va.reshape(8, 128, H, HD + 1).transpose(1, 2, 0, 3))
        in_maps.append({"qt": qtc, "kt": ktc, "vv": vvc, "wht": wht,
                        "bias": bias})
    return in_maps


def _get_runner():
    """Build (once) and cache a jitted 8-core runner for the kernel."""
    if "runner" in _STATE:
        return _STATE["runner"]

    import jax
    from jax.sharding import Mesh, PartitionSpec, NamedSharding
    from jax.experimental.shard_map import shard_map
    from concourse import bass2jax, mybir

    nc = _build_nc()
    bass2jax.install_neuronx_cc_hook()

    partition_name = (
        nc.partition_id_tensor.name if nc.partition_id_tensor else None)
    in_names, out_names, out_avals, zero_shapes = [], [], [], []
    for alloc in nc.m.functions[0].allocations:
        if not isinstance(alloc, mybir.MemoryLocationSet):
            continue
        name = alloc.memorylocations[0].name
        if alloc.kind == "ExternalInput":
            if name != partition_name:
                in_names.append(name)
        elif alloc.kind == "ExternalOutput":
            out_names.append(name)
            shape = tuple(alloc.tensor_shape)
            dtype = mybir.dt.np(alloc.dtype)
            out_avals.append(jax.core.ShapedArray(shape, dtype))
            zero_shapes.append((shape, dtype))
    n_params = len(in_names)
    n_outs = len(out_avals)
    all_names = list(in_names) + list(out_names)
    if partition_name is not None:
        all_names.append(partition_name)
    donate = tuple(range(n_params, n_params + n_outs))

    def _body(*args):
        operands = list(args)
        if partition_name is not None:
            operands.append(bass2jax.partition_id_tensor())
        outs = bass2jax._bass_exec_p.bind(
            *operands,
            out_avals=tuple(out_avals),
            in_names=tuple(all_names),
            out_names=tuple(out_names),
            lowering_input_output_aliases=(),
            sim_require_finite=True,
            sim_require_nnan=True,
            nc=nc,
        )
        return tuple(outs)

    devices = jax.devices()[:NCORES]
    mesh = Mesh(np.asarray(devices), ("core",))
    in_specs = (PartitionSpec("core"),) * (n_params + n_outs)
    out_specs = (PartitionSpec("core"),) * n_outs
    sharded = jax.jit(
        shard_map(_body, mesh=mesh, in_specs=in_specs, out_specs=out_specs,
                  check_rep=False),
        donate_argnums=donate,
        keep_unused=True,
    )
    sharding = NamedSharding(mesh, PartitionSpec("core"))

    def put_inputs(in_maps):
        return [
            jax.device_put(
                np.concatenate(
                    [np.asarray(in_maps[c][nm]) for c in range(NCORES)], axis=0),
                sharding)
            for nm in in_names
        ]

    def run(in_maps, device_inputs=None):
        if device_inputs is None:
            device_inputs = put_inputs(in_maps)
        zeros = [
            jax.device_put(np.zeros((NCORES * s[0], *s[1:]), d), sharding)
            for s, d in zero_shapes
        ]
        out_arrs = sharded(*device_inputs, *zeros)
        results = []
        for c in range(NCORES):
            results.append({
                name: np.asarray(out_arrs[i]).reshape(
                    NCORES, *out_avals[i].shape)[c]
                for i, name in enumerate(out_names)
            })
        return results

    runner = {"run": run, "put_inputs": put_inputs, "sharded": sharded,
              "in_names": in_names, "out_names": out_names,
              "zero_shapes": zero_shapes, "nc": nc}
    _STATE["runner"] = runner
    return runner


def kernel(Q=None, K=None, V=None, W_H=None, b_H=None, mask=None, **kw):
    in_maps = _host_shard(Q, K, V, W_H, b_H)
    runner = _get_runner()
    results = runner["run"](in_maps)
    out = np.empty((B, L, D), np.float32)
    for c in range(NCORES):
        b, half = divmod(c, 2)
        out[b, half * QBLK:(half + 1) * QBLK, :] = results[c]["out"]
    return out
